# revision 7
# baseline (speedup 1.0000x reference)
"""Trainium2 Bass kernel: CNN encoder (conv1d F=8, D=128 -> K=256, valid, + bias + ReLU).

Computation: out[b, l, k] = relu(b_k[k] + sum_{f,d} x[b, l+f, d] * filt[f,d] * W[f*D+d, k])
for l in [0, L-F)  (2040 windows).

Strategy:
  - Data-parallel: 32 batches / 8 cores = 4 batches per core. Params replicated.
  - Host folds filt into W (Wp[f,d,k] = filt[f,d]*W[f*128+d,k]) and transposes x to
    d-major (xT[b, d, l]) so the contraction dim (d=128) lands on SBUF partitions
    with fully-contiguous DMA.
  - On device: for each 512-wide stripe of output positions l and each half of k,
    accumulate 8 matmuls (one per filter tap f) into one PSUM bank:
        psum[k=128p, l=512] += Wp[f,:,kh].T @ xT[:, l0+f : l0+f+512]
    using float32r (full-rate single-pass fp32 matmul; moving dim 512 >= 256).
  - Eviction fuses bias-add + ReLU in one op (bias is per-partition since k is the
    partition dim), alternating ScalarE activation / VectorE tensor_scalar.
  - Output written k-major ([b, k, l]); host transposes back to [b, l, k].
"""

import os

import numpy as np

import concourse.bacc as bacc
import concourse.bass as bass
import concourse.tile as tile
import concourse.mybir as mybir
from concourse.bass_utils import run_bass_kernel_spmd

if os.environ.get("LDW_OPT") == "1":
    # experiment: let walrus optimize the per-matmul fused weight loads
    from concourse import bass_utils as _bu
    if not getattr(_bu, "_ldw_opt_patched", False):
        _orig_run_command = _bu.run_command

        def _patched_run_command(argv, **kw):
            argv = ["--enable-ldw-opt=true" if a == "--enable-ldw-opt=false" else a
                    for a in argv]
            return _orig_run_command(argv, **kw)

        _bu.run_command = _patched_run_command
        _bu._ldw_opt_patched = True

F32 = mybir.dt.float32
F32R = mybir.dt.float32r

N_CORES = 8
B, L, D = 32, 2048, 128
F, K = 8, 256
N_WIN = L - F            # 2040
BP = B // N_CORES        # batches per core
KH = K // 128            # k halves
# output-position stripes per batch: 3x512 + 1x504
SUPERS = [(0, 512), (512, 512), (1024, 512), (1536, N_WIN - 1536)]

# matmul input dtype: F32R = single-pass fp32 (full PE rate at N>=256), F32 = 2-pass
MM_DT = F32R

# PAIR=1 interleaves the two matmul accumulation groups of a super-pair with
# shared stationary weights. It produces correct single-shot results but has
# faulted the device (NRT_EXEC_UNIT_UNRECOVERABLE) under sustained looping, so
# the safe sequential schedule is the default.
PAIR = os.environ.get("PAIR", "0") == "1"

# weight dtype for the stationary operand: f32r (precise) or bf16 (fast load)
W_DT = mybir.dt.bfloat16 if os.environ.get("WDT", "f32r") == "bf16" else F32R
# activation (moving operand) dtype — hardware forbids mixing 32/16-bit, so
# bf16 weights require bf16 activations too
X_DT = mybir.dt.bfloat16 if os.environ.get("WDT", "f32r") == "bf16" else F32R

# SCHED=fouter: per (half, kh) group, loop f OUTER over all 8 PSUM banks
# (4 batches x 2 supers) so each stationary weight is loaded once per 8
# matmuls (4096 moving cols) instead of once per matmul (512 cols).
SCHED = os.environ.get("SCHED", "base")


def _build_program(reps=1, loop_n=0):
    """One SPMD program for all 8 cores. reps>1 unrolls the full body (input
    DMAs + compute + output DMAs); rep r writes to output rows [r*BP, (r+1)*BP).
    loop_n>0 additionally wraps the body in a hardware For_i loop (benchmarking
    only: every loop iteration rewrites the same output region)."""
    nc = bacc.Bacc(
        "TRN2",
        target_bir_lowering=False,
        debug=False,
        num_devices=N_CORES,
    )
    xT_d = nc.declare_dram_parameter("xT", [BP, D, L], X_DT, isOutput=False)
    wp_d = nc.declare_dram_parameter("wp", [KH, D, F, 128], W_DT, isOutput=False)
    bias_d = nc.declare_dram_parameter("bias", [128, KH], F32, isOutput=False)
    # bench-only unrolling (reps>1) re-writes the same BP output rows so the
    # download size stays constant
    out_d = nc.declare_dram_parameter(
        "outT", [BP, KH, 128, N_WIN], F32, isOutput=True)

    # xt halves: lo covers l in [0, HALF+F), hi covers [HALF, L). Supers 0-1
    # read only lo, supers 2-3 only hi, so each matmul depends on exactly one
    # half-tile DMA (~516KB) instead of the full 1MB batch load.
    HALF = L // 2
    LO_W = HALF + F  # 1032

    # out-DMA stream points: after super si, DMA ob columns [lo, hi)
    OUT_CHUNKS = {1: (0, 1024), 2: (1024, 1536), 3: (1536, N_WIN)}

    def body(nc, tc, pools, r, warm):
        const_pool, xt_pool, psum_pool, out_pool = pools
        bias_sb = const_pool.tile([128, KH], F32, tag="bias")
        wp_sb = []
        for kh in range(KH):
            t_wp = const_pool.tile([D, F, 128], W_DT, tag=f"wp{kh}")
            wp_sb.append(t_wp)

        xt_lo, xt_hi = [], []
        for b in range(BP):
            t_lo = xt_pool.tile([D, LO_W], X_DT, tag="xtlo")
            t_hi = xt_pool.tile([D, L - HALF], X_DT, tag="xthi")
            xt_lo.append(t_lo)
            xt_hi.append(t_hi)

        if warm:
            # PE HAM warm-up on junk data while the first input DMAs land.
            # Plain fp32 matmuls (4 cycles/row, ~850ns each) keep the PE busy
            # through the ~3.5us clock-gate window without f32r's
            # rounded-producer requirement.
            warm_x = const_pool.tile([D, 64], F32, tag="warmx")
            warm_ps = psum_pool.tile([128, 512], F32, tag="ps")
            nc.gpsimd.memset(warm_x[:], 0.0)
            for _ in range(16):
                nc.tensor.matmul(warm_ps[0:64, 0:64], lhsT=warm_x[:, 0:64],
                                 rhs=warm_x[:], start=True, stop=True)

        # issue order: batch-0 lo + first weight half first so compute starts ASAP
        nc.sync.dma_start(xt_lo[0][:], xT_d[0, :, 0:LO_W])
        nc.sync.dma_start(wp_sb[0][:], wp_d[0])
        nc.sync.dma_start(bias_sb[:], bias_d[:])
        nc.sync.dma_start(wp_sb[1][:], wp_d[1])
        nc.sync.dma_start(xt_hi[0][:], xT_d[0, :, HALF:L])
        for b in range(1, BP):
            nc.sync.dma_start(xt_lo[b][:], xT_d[b, :, 0:LO_W])
            nc.sync.dma_start(xt_hi[b][:], xT_d[b, :, HALF:L])

        # Supers are processed in pairs (s0,s1) and (s2,s3): the f-loop issues
        # the pair's two matmuls back-to-back with the SAME stationary weights
        # so the PE can skip/overlap redundant weight loads.
        evictor = 0
        for b in range(BP):
            for kh in range(KH):
                ob = out_pool.tile([128, N_WIN], F32, tag="ob")
                for pair in range(2):
                    xt = xt_lo[b] if pair == 0 else xt_hi[b]
                    sis = (2 * pair, 2 * pair + 1)
                    pss, bases, lss = [], [], []
                    for si in sis:
                        l0, ls = SUPERS[si]
                        ps = psum_pool.tile([128, 512], F32, tag="ps")
                        pss.append(ps)
                        bases.append(l0 if pair == 0 else l0 - HALF)
                        lss.append(ls)
                    if PAIR:
                        for f in range(F):
                            for j in range(2):
                                nc.tensor.matmul(
                                    pss[j][:, :lss[j]],
                                    lhsT=wp_sb[kh][:, f, :],
                                    rhs=xt[:, bases[j] + f:bases[j] + f + lss[j]],
                                    start=(f == 0),
                                    stop=(f == F - 1),
                                )
                    else:
                        for j in range(2):
                            for f in range(F):
                                nc.tensor.matmul(
                                    pss[j][:, :lss[j]],
                                    lhsT=wp_sb[kh][:, f, :],
                                    rhs=xt[:, bases[j] + f:bases[j] + f + lss[j]],
                                    start=(f == 0),
                                    stop=(f == F - 1),
                                )
                    for j, si in enumerate(sis):
                        l0, ls = SUPERS[si]
                        if evictor == 0:
                            nc.scalar.activation(
                                ob[:, l0:l0 + ls], pss[j][:, :ls],
                                mybir.ActivationFunctionType.Relu,
                                bias=bias_sb[:, kh:kh + 1], scale=1.0,
                            )
                        else:
                            nc.vector.tensor_scalar(
                                ob[:, l0:l0 + ls], pss[j][:, :ls],
                                scalar1=bias_sb[:, kh:kh + 1], scalar2=0.0,
                                op0=mybir.AluOpType.add, op1=mybir.AluOpType.max,
                            )
                        evictor ^= 1
                        if si in OUT_CHUNKS:
                            lo, hi = OUT_CHUNKS[si]
                            nc.sync.dma_start(out_d[b, kh, :, lo:hi],
                                              ob[:, lo:hi])

    def body_fouter(nc, tc, pools, r, warm):
        const_pool, xt_pool, psum_pool, out_pool = pools
        bias_sb = const_pool.tile([128, KH], F32, tag="bias")
        wp_sb = []
        for kh in range(KH):
            t_wp = const_pool.tile([D, F, 128], W_DT, tag=f"wp{kh}")
            wp_sb.append(t_wp)

        xt_lo, xt_hi = [], []
        for b in range(BP):
            t_lo = xt_pool.tile([D, LO_W], X_DT, tag="xtlo")
            t_hi = xt_pool.tile([D, L - HALF], X_DT, tag="xthi")
            xt_lo.append(t_lo)
            xt_hi.append(t_hi)

        if warm:
            warm_x = const_pool.tile([D, 64], F32, tag="warmx")
            warm_ps = psum_pool.tile([128, 512], F32, tag="ps")
            nc.gpsimd.memset(warm_x[:], 0.0)
            for _ in range(16):
                nc.tensor.matmul(warm_ps[0:64, 0:64], lhsT=warm_x[:, 0:64],
                                 rhs=warm_x[:], start=True, stop=True)

        nc.sync.dma_start(xt_lo[0][:], xT_d[0, :, 0:LO_W])
        nc.sync.dma_start(wp_sb[0][:], wp_d[0])
        nc.sync.dma_start(bias_sb[:], bias_d[:])
        nc.sync.dma_start(wp_sb[1][:], wp_d[1])
        for b in range(1, BP):
            nc.sync.dma_start(xt_lo[b][:], xT_d[b, :, 0:LO_W])
        for b in range(BP):
            nc.sync.dma_start(xt_hi[b][:], xT_d[b, :, HALF:L])

        evictor = 0
        for half in range(2):
            xt = xt_lo if half == 0 else xt_hi
            sis = (2 * half, 2 * half + 1)
            ob_w = sum(SUPERS[si][1] for si in sis)   # 1024 (lo) / 1016 (hi)
            for kh in range(KH):
                pss = [psum_pool.tile([128, 512], F32, tag="ps")
                       for _ in range(2 * BP)]
                for f in range(F):
                    for b in range(BP):
                        for s, si in enumerate(sis):
                            l0, ls = SUPERS[si]
                            base = l0 - half * HALF
                            nc.tensor.matmul(
                                pss[2 * b + s][:, :ls],
                                lhsT=wp_sb[kh][:, f, :],
                                rhs=xt[b][:, base + f:base + f + ls],
                                start=(f == 0),
                                stop=(f == F - 1),
                            )
                for b in range(BP):
                    ob = out_pool.tile([128, 1024], F32, tag="ob")
                    for s, si in enumerate(sis):
                        l0, ls = SUPERS[si]
                        dst = ob[:, s * 512:s * 512 + ls]
                        src = pss[2 * b + s][:, :ls]
                        if evictor == 0:
                            nc.scalar.activation(
                                dst, src, mybir.ActivationFunctionType.Relu,
                                bias=bias_sb[:, kh:kh + 1], scale=1.0)
                        else:
                            nc.vector.tensor_scalar(
                                dst, src,
                                scalar1=bias_sb[:, kh:kh + 1], scalar2=0.0,
                                op0=mybir.AluOpType.add, op1=mybir.AluOpType.max)
                        evictor ^= 1
                    nc.sync.dma_start(
                        out_d[b, kh, :, half * HALF:half * HALF + ob_w],
                        ob[:, :ob_w])

    body_fn = body_fouter if SCHED == "fouter" else body
    n_psum = 8 if SCHED == "fouter" else 6
    n_out = 8 if SCHED == "fouter" else 4

    with tile.TileContext(nc) as tc:
        with (
            tc.tile_pool(name="const", bufs=2) as const_pool,
            tc.tile_pool(name="xt", bufs=BP) as xt_pool,
            tc.tile_pool(name="psum", bufs=n_psum, space=bass.MemorySpace.PSUM) as psum_pool,
            tc.tile_pool(name="out", bufs=n_out) as out_pool,
        ):
            pools = (const_pool, xt_pool, psum_pool, out_pool)
            if loop_n > 0:
                with tc.For_i(0, loop_n, 1,
                              hint_engines=(mybir.EngineType.PE,)):
                    for r in range(reps):
                        body_fn(nc, tc, pools, r, warm=(r == 0))
            else:
                for r in range(reps):
                    body_fn(nc, tc, pools, r, warm=(r == 0))
    nc.compile()
    return nc


def _prep_inputs(user_batch, filt, W_k, b_k):
    user_batch = np.asarray(user_batch, dtype=np.float32)
    filt = np.asarray(filt, dtype=np.float32)
    W_k = np.asarray(W_k, dtype=np.float32)
    b_k = np.asarray(b_k, dtype=np.float32)

    wp = W_k.reshape(F, D, K) * filt[:, :, None]          # [f, d, k]
    wp_host = np.ascontiguousarray(                        # [kh, d, f, 128]
        wp.reshape(F, D, KH, 128).transpose(2, 1, 0, 3))
    bias_host = np.ascontiguousarray(b_k.reshape(KH, 128).T)  # [128, kh]
    xT = np.ascontiguousarray(user_batch.transpose(0, 2, 1))  # [b, d, l]
    if W_DT == mybir.dt.bfloat16:
        import ml_dtypes
        wp_host = wp_host.astype(ml_dtypes.bfloat16)
        xT = xT.astype(ml_dtypes.bfloat16)
    return xT, wp_host, bias_host


def _make_in_maps(xT, wp_host, bias_host):
    return [
        {"xT": xT[c * BP:(c + 1) * BP], "wp": wp_host, "bias": bias_host}
        for c in range(N_CORES)
    ]


def _unshard(res):
    outT = np.concatenate([r["outT"] for r in res.results], axis=0)  # [B, KH, 128, N_WIN]
    out = outT.reshape(B, K, N_WIN).transpose(0, 2, 1)               # [B, N_WIN, K]
    return np.ascontiguousarray(out)


def _run(user_batch, filt, W_k, b_k, trace=False):
    xT, wp_host, bias_host = _prep_inputs(user_batch, filt, W_k, b_k)
    nc = _build_program()
    in_maps = _make_in_maps(xT, wp_host, bias_host)
    res = run_bass_kernel_spmd(nc, in_maps, list(range(N_CORES)), trace=trace)
    return _unshard(res), res


def kernel(user_batch, filt, W_k, b_k):
    out, _ = _run(user_batch, filt, W_k, b_k, trace=False)
    return out

